# revision 24
# baseline (speedup 1.0000x reference)
"""RWKV-style AttentionBlock kernel for 8 Trainium2 NeuronCores (v3).

Problem: B=8, T=4096, D=1024, f32 in/out.
  per sequence: k/v/r = token-shift-mixed x @ W{k,v,r}.T ; imp = exp(k)
  WKV linear recurrence over time (per-channel decay), bonus-gain readout,
  rwkv = sigmoid(r) * wkv ; out = rwkv @ Wo.T

Sharding: pure data-parallel, one batch element per core (no collectives).

Measured engine economics (HW traces):
  - DVE: tt 413ns, stt 732ns (no fast mode), scan 1272ns per [128,512]
  - ACT: ~693ns per [128,512] op
  - PE fp16 matmul [128ctr,512]: 213ns; fp8e4 DoubleRow [256ctr,512]: ~250ns
    (i.e. DR is ~2x flops/s of fp16 -- NOT the 4x the cost model claims)
  - TRN fp8e4 is IEEE float8_e4m3: max normal 240 (not 448!)

Design:
  - K and R projections run as fp8 DoubleRow GEMMs (half PE cost); their
    quantization error survives the WKV ratio / sigmoid (~1.1e-2 total,
    gate 2e-2). V and O stay fp16 (value path is 3.7e-2 if fp8).
  - token-shift mixes produce the GEMM inputs: xk8/xr8 fp8 straight out
    of the stt (stt has no fast mode, so fp8 output is free), xv16 fp16.
  - mixes + diff optionally run on the idle GpSimd (Pool) engine
    (GPS_MIX=1) to unload the bottleneck DVE.
  - u = imp*v and rwkv = num*recip as single [128, 8*TC] fp16 tt ops;
    ln/exp reciprocal batched on ACT; scan state planes updated in place
    (DVE is in-order).
  - one-chunk software pipelining: mixes run a chunk ahead of the GEMMs;
    the rwkv mul + O GEMM of chunk ch runs during chunk ch+1.
"""

import os
import numpy as np
from contextlib import ExitStack

import ml_dtypes

import concourse.mybir as mybir
import concourse.tile as tile
from concourse import bacc
from concourse.bass_utils import run_bass_kernel_spmd

# ---------------------------------------------------------------------------
# Pin Exp/Ln to the one ACT table set holding both (avoids ~1.3us table
# reloads between exp and ln on the scalar engine).
import concourse.hw_specs as _hw_specs

_orig_get_activation_tables = _hw_specs.get_activation_tables


def _pinned_activation_tables(arch):
    tabs = _orig_get_activation_tables(arch)
    AF_ = mybir.ActivationFunctionType
    both = [n for n, fs in tabs.items() if AF_.Exp in fs and AF_.Ln in fs]
    if both:
        keep = both[0]
        for n, fs in tabs.items():
            if n != keep:
                fs.discard(AF_.Exp)
                fs.discard(AF_.Ln)
    return tabs


if os.environ.get("PIN_ACT_TABLES", "1") == "1":
    _hw_specs.get_activation_tables = _pinned_activation_tables
    bacc.get_activation_tables = _pinned_activation_tables

P = 128
D = 1024
DT = D // P          # 8 channel tiles
B = 8
T_FULL = 4096
TC_DEFAULT = 512

F16 = mybir.dt.float16
F32 = mybir.dt.float32
F8 = mybir.dt.float8e4
E4NP = ml_dtypes.float8_e4m3  # IEEE e4m3: max normal 240
PPDT = F32  # fp16 per-partition scalars deadlock the DVE on hw; keep f32
AL = mybir.AluOpType
AF = mybir.ActivationFunctionType
DR = mybir.MatmulPerfMode.DoubleRow

SX = 32.0     # x (and mixed x) scale into fp8: |x|max ~5.5 -> 176 < 240
SW = 1024.0   # weight scale into fp8: |W|max ~0.11 -> ~115 < 240
KSCALE = 1.0 / (SX * SW)

# run the pure tensor_tensor planes (diff, u, rwkv-mul) on GpSimd (Pool);
# stt (mixes) is not a valid Pool opcode on corev3, so those stay on DVE
GPS_TT = os.environ.get("GPS_TT", "1") == "1"
# probe: materialized decay tile for the dt0 scans (vs broadcast stride-0)
SCAN_PROBE = os.environ.get("SCAN_PROBE", "1") == "1"


def build(T=T_FULL, TC=TC_DEFAULT):
    assert T % TC == 0
    NCH = T // TC
    nc = bacc.Bacc("TRN2", target_bir_lowering=False, debug=False, num_devices=B)

    x_d = nc.dram_tensor("x", [P, DT, T], F16, kind="ExternalInput")
    wk_d = nc.dram_tensor("wk", [P, DT, D], F8, kind="ExternalInput")
    wv_d = nc.dram_tensor("wv", [P, DT, D], F16, kind="ExternalInput")
    wr_d = nc.dram_tensor("wr", [P, DT, D], F8, kind="ExternalInput")
    wo_d = nc.dram_tensor("wo", [P, DT, D], F16, kind="ExternalInput")
    # per-channel params, packed [128, DT, 8]: mix_k, mix_v, mix_r, decay, gain
    pp_d = nc.dram_tensor("pp", [P, DT, 8], PPDT, kind="ExternalInput")
    out_d = nc.dram_tensor("out", [P, DT, T], F16, kind="ExternalOutput")

    mixer = None  # set inside context

    with tile.TileContext(nc) as tc, ExitStack() as ctx:
        const = ctx.enter_context(tc.tile_pool(name="const", bufs=1))
        xpool = ctx.enter_context(tc.tile_pool(name="xpool", bufs=2))
        mixp = ctx.enter_context(tc.tile_pool(name="mixp", bufs=2))
        diffp = ctx.enter_context(tc.tile_pool(name="diffp", bufs=2))
        pl2 = ctx.enter_context(tc.tile_pool(name="pl2", bufs=2))
        pl1 = ctx.enter_context(tc.tile_pool(name="pl1", bufs=1))
        outp = ctx.enter_context(tc.tile_pool(name="outp", bufs=1))
        psp = ctx.enter_context(tc.tile_pool(name="psp", bufs=5, space="PSUM"))
        pso = ctx.enter_context(tc.tile_pool(name="pso", bufs=3, space="PSUM"))

        pp_sb = const.tile([P, DT, 8], PPDT, tag="pp")
        nc.sync.dma_start(pp_sb[:], pp_d[:])
        xt0 = xpool.tile([P, DT, TC + 1], F16, tag="xt", name="xt0")
        for dt_i in range(DT):
            nc.vector.memset(xt0[:, dt_i, 0:1], 0.0)
        nc.sync.dma_start(xt0[:, :, 1:], x_d[:, :, 0:TC])
        w_sb = {}
        for nm, dram, dt_ in (("k", wk_d, F8), ("v", wv_d, F16),
                              ("r", wr_d, F8), ("o", wo_d, F16)):
            w = const.tile([P, DT, D], dt_, tag=f"w{nm}")
            nc.sync.dma_start(w[:], dram[:])
            w_sb[nm] = w

        def pc(dt_i, j):
            return pp_sb[:, dt_i, j : j + 1]

        tte = nc.gpsimd if GPS_TT else nc.vector

        def make_mixes(xt):
            """diff (Pool-able tt) + 3 token-shift mixes (DVE stt);
            fp8 out for k/r, fp16 for v."""
            xk8 = mixp.tile([P, DT, TC], F8, tag="xk8")
            xr8 = mixp.tile([P, DT, TC], F8, tag="xr8")
            xv16 = mixp.tile([P, DT, TC], F16, tag="xv16")
            for dt_i in range(DT):
                diff = diffp.tile([P, TC], F16, tag="diff")
                tte.tensor_sub(diff[:], xt[:, dt_i, 1:], xt[:, dt_i, 0:TC])
                nc.vector.scalar_tensor_tensor(
                    xk8[:, dt_i, :], diff[:], pc(dt_i, 0), xt[:, dt_i, 0:TC],
                    AL.mult, AL.add)
                nc.vector.scalar_tensor_tensor(
                    xv16[:, dt_i, :], diff[:], pc(dt_i, 1), xt[:, dt_i, 0:TC],
                    AL.mult, AL.add)
                nc.vector.scalar_tensor_tensor(
                    xr8[:, dt_i, :], diff[:], pc(dt_i, 2), xt[:, dt_i, 0:TC],
                    AL.mult, AL.add)
            return xk8, xr8, xv16

        # persistent scan-state planes (chunk ch init reads the last column
        # written by chunk ch-1; DVE is in-order so in-place is safe)
        c_pl = pl1.tile([P, DT, TC], F16, tag="c_pl")
        n_pl = pl1.tile([P, DT, TC], F16, tag="n_pl")

        dk_mat = None
        if SCAN_PROBE:
            # materialized decay tile for dt0: is the stride-0 broadcast
            # data0 what makes hw scans ~2.4x the cost model?
            dk_mat = const.tile([P, TC], F16, tag="dk_mat")
            nc.scalar.activation(dk_mat[:], pc(0, 3).to_broadcast((P, TC)),
                                 AF.Copy)

        def dma_x(ch_i):
            xt = xpool.tile([P, DT, TC + 1], F16, tag="xt")
            nc.sync.dma_start(xt[:], x_d[:, :, ch_i * TC - 1 : (ch_i + 1) * TC])
            return xt

        mixes = make_mixes(xt0)  # chunk 0, pipeline warm-up
        xt_next = dma_x(1) if NCH > 1 else None
        pend = None

        for ch in range(NCH):
            t0 = ch * TC
            xk8, xr8, xv16 = mixes

            # two-deep prefetch: DMA ch+2 now, mix ch+1 (DMA'd last iter)
            if ch + 2 < NCH:
                xt_next2 = dma_x(ch + 2)
            else:
                xt_next2 = None
            if xt_next is not None:
                mixes = make_mixes(xt_next)
            xt_next = xt_next2

            imp = pl2.tile([P, DT, TC], F16, tag="imp")
            v16 = pl2.tile([P, DT, TC], F16, tag="v16")
            er = pl2.tile([P, DT, TC], F16, tag="er")

            # ---- K/R fp8 DoubleRow GEMMs + V fp16 GEMM
            for dt_i in range(DT):
                cs = slice(dt_i * P, (dt_i + 1) * P)

                ps_k = psp.tile([P, TC], F32, tag="ps")
                for j in range(DT // 2):
                    nc.tensor.matmul(
                        ps_k[:], w_sb["k"][:, 2 * j : 2 * j + 2, cs],
                        xk8[:, 2 * j : 2 * j + 2, :],
                        start=(j == 0), stop=(j == DT // 2 - 1), perf_mode=DR)
                nc.scalar.activation(imp[:, dt_i, :], ps_k[:], AF.Exp,
                                     scale=KSCALE)

                ps_v = psp.tile([P, TC], F32, tag="ps")
                for eo in range(DT):
                    nc.tensor.matmul(
                        ps_v[:], w_sb["v"][:, eo, cs], xv16[:, eo, :],
                        start=(eo == 0), stop=(eo == DT - 1))
                nc.scalar.copy(v16[:, dt_i, :], ps_v[:])

                ps_r = psp.tile([P, TC], F32, tag="ps")
                for j in range(DT // 2):
                    nc.tensor.matmul(
                        ps_r[:], w_sb["r"][:, 2 * j : 2 * j + 2, cs],
                        xr8[:, 2 * j : 2 * j + 2, :],
                        start=(j == 0), stop=(j == DT // 2 - 1), perf_mode=DR)
                nc.scalar.activation(er[:, dt_i, :], ps_r[:], AF.Exp,
                                     scale=-KSCALE)

            # ---- WKV chain on DVE
            u = pl1.tile([P, DT, TC], F16, tag="u")
            tte.tensor_mul(u[:], imp[:], v16[:])

            for dt_i in range(DT):
                if SCAN_PROBE and dt_i == 0:
                    decay_b = dk_mat[:]
                else:
                    decay_b = pc(dt_i, 3).to_broadcast((P, TC))
                init_c = 0.0 if ch == 0 else c_pl[:, dt_i, TC - 1 : TC]
                init_n = 0.0 if ch == 0 else n_pl[:, dt_i, TC - 1 : TC]
                nc.vector.tensor_tensor_scan(
                    c_pl[:, dt_i, :], decay_b, u[:, dt_i, :], init_c,
                    AL.mult, AL.add)
                nc.vector.tensor_tensor_scan(
                    n_pl[:, dt_i, :], decay_b, imp[:, dt_i, :], init_n,
                    AL.mult, AL.add)

            # ---- drain previous chunk's tail (its ACT ln/exp is done; sits
            # between scans and stts so the DVE never stalls on the ACT)
            if pend is not None:
                _flush_tail(nc, tte, pso, outp, out_d, w_sb["o"], pend)
                pend = None

            num = pl1.tile([P, DT, TC], F16, tag="num")
            den2 = pl1.tile([P, DT, TC], F16, tag="den2")
            for dt_i in range(DT):
                nc.vector.scalar_tensor_tensor(
                    num[:, dt_i, :], u[:, dt_i, :], pc(dt_i, 4),
                    c_pl[:, dt_i, :], AL.mult, AL.add)
                # u[dt_i] is dead after num's stt (DVE in-order): reuse as den
                nc.vector.scalar_tensor_tensor(
                    u[:, dt_i, :], imp[:, dt_i, :], pc(dt_i, 4),
                    n_pl[:, dt_i, :], AL.mult, AL.add)
                # den2 = (er + 1) * den
                nc.vector.scalar_tensor_tensor(
                    den2[:, dt_i, :], er[:, dt_i, :], 1.0, u[:, dt_i, :],
                    AL.add, AL.mult)

            # ---- batched reciprocal on ACT: den2 -> 1/den2 (in place)
            nc.scalar.activation(den2[:, :, :], den2[:, :, :], AF.Ln)
            nc.scalar.activation(den2[:, :, :], den2[:, :, :], AF.Exp,
                                 scale=-1.0)

            pend = (num, den2, ch)

        _flush_tail(nc, tte, pso, outp, out_d, w_sb["o"], pend)

    nc.compile()
    return nc


def _flush_tail(nc, tte, pso, outp, out_d, wo_sb, pend):
    num, recip, ch = pend
    TC = num.shape[2]
    t0 = ch * TC
    rwkv = outp.tile([P, DT, TC], F16, tag="rwkv")
    tte.tensor_mul(rwkv[:], num[:], recip[:])
    out16 = outp.tile([P, DT, TC], F16, tag="out16")
    for co in range(DT):
        cs = slice(co * P, (co + 1) * P)
        ps_o = pso.tile([P, TC], F32, tag="pso")
        for eo in range(DT):
            nc.tensor.matmul(
                ps_o[:], wo_sb[:, eo, cs], rwkv[:, eo, :],
                start=(eo == 0), stop=(eo == DT - 1))
        nc.scalar.copy(out16[:, co, :], ps_o[:])
    nc.sync.dma_start(out_d[:, :, t0 : t0 + TC], out16[:])


def _pack_vec(v):
    # [D] -> [P, DT]
    return np.ascontiguousarray(v.reshape(DT, P).T)


def _packw_T(W):
    # W [c, e] -> W.T [e, c] -> [P, DT, D]
    return np.ascontiguousarray(W.T.reshape(DT, P, D).transpose(1, 0, 2))


def pack_inputs(x, Wk, Wv, Wr, Wo, mix_k, mix_v, mix_r, log_gain, log_decay):
    T = x.shape[1]
    decay = np.exp(-np.exp(log_decay.astype(np.float64))).astype(np.float32)
    gain = (np.exp(log_gain.astype(np.float64)) - 1.0).astype(np.float32)
    # mix scalars are applied to fp16 x, producing fp8 (k/r) pre-scaled by
    # SX: fold SX into the mix coefficients? No -- SX folds into x itself
    # for the k/r paths via the stt scalar... the stt computes
    # (diff*mix + x_prev); scaling by SX must hit both terms, so scale x.
    # Instead: x stays unscaled fp16; the fp8 quantization step of the stt
    # output loses range for small values. Scale mix path by folding SX
    # into the *weights* being fp8 is not possible (they're also fp8).
    # Resolution: ship x pre-scaled by SX (fp16 holds 5.5*32=176 fine) and
    # de-scale in the V GEMM via the fp16 weights (Wv/SX) and in KSCALE.
    pp = np.zeros((P, DT, 8), np.float32)
    for j, v in enumerate((mix_k, mix_v, mix_r, decay, gain)):
        pp[:, :, j] = _pack_vec(v.astype(np.float32))

    wk8 = _packw_T((Wk.astype(np.float64) * SW)).astype(E4NP)
    wr8 = _packw_T((Wr.astype(np.float64) * SW)).astype(E4NP)
    # x is shipped *SX; compensate inside the fp16 V and... V reads x*SX,
    # so Wv scales by 1/SX. O reads rwkv (unscaled).
    wv16 = _packw_T((Wv.astype(np.float64) / SX)).astype(np.float16)
    wo16 = _packw_T(Wo).astype(np.float16)

    xs = (x.astype(np.float64) * SX).astype(np.float16)

    in_maps = []
    for b in range(x.shape[0]):
        xb = np.ascontiguousarray(
            xs[b].T.reshape(DT, P, T).transpose(1, 0, 2))
        in_maps.append({"x": xb, "wk": wk8, "wv": wv16, "wr": wr8,
                        "wo": wo16, "pp": pp})
    return in_maps


def unpack_output(arrs, T):
    out = np.empty((len(arrs), T, D), np.float32)
    for b, a in enumerate(arrs):
        out[b] = a.astype(np.float32).transpose(2, 1, 0).reshape(T, D)
    return out


_NC_CACHE = {}


def run(inputs, trace=False, **kw):
    x = np.asarray(inputs["x"])
    Bx, T, Dx = x.shape
    assert Dx == D and Bx == B
    key = (T, TC_DEFAULT, GPS_TT, SCAN_PROBE)
    if key not in _NC_CACHE:
        _NC_CACHE[key] = build(T=T)
    nc = _NC_CACHE[key]
    in_maps = pack_inputs(
        x,
        np.asarray(inputs["Wk"]), np.asarray(inputs["Wv"]),
        np.asarray(inputs["Wr"]), np.asarray(inputs["Wo"]),
        np.asarray(inputs["mix_k"]), np.asarray(inputs["mix_v"]),
        np.asarray(inputs["mix_r"]),
        np.asarray(inputs["log_gain"]), np.asarray(inputs["log_decay"]),
    )
    res = run_bass_kernel_spmd(nc, in_maps, core_ids=list(range(B)), trace=trace, **kw)
    out = unpack_output([res.results[i]["out"] for i in range(B)], T)
    return out, res


def kernel(**inputs):
    return run(inputs)[0]


if __name__ == "__main__":
    nc = build(T=512)
    print("built ok")


# revision 27
# speedup vs baseline: 1.1567x; 1.1567x over previous
"""RWKV-style AttentionBlock kernel for 8 Trainium2 NeuronCores (v3).

Problem: B=8, T=4096, D=1024, f32 in/out.
  per sequence: k/v/r = token-shift-mixed x @ W{k,v,r}.T ; imp = exp(k)
  WKV linear recurrence over time (per-channel decay), bonus-gain readout,
  rwkv = sigmoid(r) * wkv ; out = rwkv @ Wo.T

Sharding: pure data-parallel, one batch element per core (no collectives).

Measured engine economics (HW traces):
  - DVE: tt 413ns, stt 732ns (no fast mode), scan 1272ns per [128,512]
  - ACT: ~693ns per [128,512] op
  - PE fp16 matmul [128ctr,512]: 213ns; fp8e4 DoubleRow [256ctr,512]: ~250ns
    (i.e. DR is ~2x flops/s of fp16 -- NOT the 4x the cost model claims)
  - TRN fp8e4 is IEEE float8_e4m3: max normal 240 (not 448!)

Design:
  - K and R projections run as fp8 DoubleRow GEMMs (half PE cost); their
    quantization error survives the WKV ratio / sigmoid (~1.1e-2 total,
    gate 2e-2). V and O stay fp16 (value path is 3.7e-2 if fp8).
  - token-shift mixes produce the GEMM inputs: xk8/xr8 fp8 straight out
    of the stt (stt has no fast mode, so fp8 output is free), xv16 fp16.
  - mixes + diff optionally run on the idle GpSimd (Pool) engine
    (GPS_MIX=1) to unload the bottleneck DVE.
  - u = imp*v and rwkv = num*recip as single [128, 8*TC] fp16 tt ops;
    ln/exp reciprocal batched on ACT; scan state planes updated in place
    (DVE is in-order).
  - one-chunk software pipelining: mixes run a chunk ahead of the GEMMs;
    the rwkv mul + O GEMM of chunk ch runs during chunk ch+1.
"""

import os
import numpy as np
from contextlib import ExitStack

import ml_dtypes

import concourse.mybir as mybir
import concourse.tile as tile
from concourse import bacc
from concourse.bass_utils import run_bass_kernel_spmd

# ---------------------------------------------------------------------------
# Pin Exp/Ln to the one ACT table set holding both (avoids ~1.3us table
# reloads between exp and ln on the scalar engine).
import concourse.hw_specs as _hw_specs

_orig_get_activation_tables = _hw_specs.get_activation_tables


def _pinned_activation_tables(arch):
    tabs = _orig_get_activation_tables(arch)
    AF_ = mybir.ActivationFunctionType
    both = [n for n, fs in tabs.items() if AF_.Exp in fs and AF_.Ln in fs]
    if both:
        keep = both[0]
        for n, fs in tabs.items():
            if n != keep:
                fs.discard(AF_.Exp)
                fs.discard(AF_.Ln)
    return tabs


if os.environ.get("PIN_ACT_TABLES", "1") == "1":
    _hw_specs.get_activation_tables = _pinned_activation_tables
    bacc.get_activation_tables = _pinned_activation_tables

P = 128
D = 1024
DT = D // P          # 8 channel tiles
B = 8
T_FULL = 4096
TC_DEFAULT = 512

F16 = mybir.dt.float16
F32 = mybir.dt.float32
F8 = mybir.dt.float8e4
E4NP = ml_dtypes.float8_e4m3  # IEEE e4m3: max normal 240
PPDT = F32  # fp16 per-partition scalars deadlock the DVE on hw; keep f32
AL = mybir.AluOpType
AF = mybir.ActivationFunctionType
DR = mybir.MatmulPerfMode.DoubleRow

SX = 32.0     # x (and mixed x) scale into fp8: |x|max ~5.5 -> 176 < 240
SW = 1024.0   # weight scale into fp8: |W|max ~0.11 -> ~115 < 240
KSCALE = 1.0 / (SX * SW)

# run the pure tensor_tensor planes (diff, u, rwkv-mul) on GpSimd (Pool);
# stt (mixes) is not a valid Pool opcode on corev3, so those stay on DVE
GPS_TT = os.environ.get("GPS_TT", "0") == "1"
# probe: materialized decay tile for the dt0 scans (vs broadcast stride-0)
SCAN_PROBE = os.environ.get("SCAN_PROBE", "0") == "1"


def build(T=T_FULL, TC=TC_DEFAULT):
    assert T % TC == 0
    NCH = T // TC
    nc = bacc.Bacc("TRN2", target_bir_lowering=False, debug=False, num_devices=B)

    x_d = nc.dram_tensor("x", [P, DT, T], F16, kind="ExternalInput")
    wk_d = nc.dram_tensor("wk", [P, DT, D], F8, kind="ExternalInput")
    wv_d = nc.dram_tensor("wv", [P, DT, D], F16, kind="ExternalInput")
    wr_d = nc.dram_tensor("wr", [P, DT, D], F8, kind="ExternalInput")
    wo_d = nc.dram_tensor("wo", [P, DT, D], F16, kind="ExternalInput")
    # per-channel params, packed [128, DT, 8]: mix_k, mix_v, mix_r, decay, gain
    pp_d = nc.dram_tensor("pp", [P, DT, 8], PPDT, kind="ExternalInput")
    out_d = nc.dram_tensor("out", [P, DT, T], F16, kind="ExternalOutput")

    mixer = None  # set inside context

    with tile.TileContext(nc) as tc, ExitStack() as ctx:
        const = ctx.enter_context(tc.tile_pool(name="const", bufs=1))
        xpool = ctx.enter_context(tc.tile_pool(name="xpool", bufs=2))
        mixp = ctx.enter_context(tc.tile_pool(name="mixp", bufs=2))
        diffp = ctx.enter_context(tc.tile_pool(name="diffp", bufs=2))
        pl2 = ctx.enter_context(tc.tile_pool(name="pl2", bufs=2))
        pl1 = ctx.enter_context(tc.tile_pool(name="pl1", bufs=1))
        outp = ctx.enter_context(tc.tile_pool(name="outp", bufs=1))
        psp = ctx.enter_context(tc.tile_pool(name="psp", bufs=5, space="PSUM"))
        pso = ctx.enter_context(tc.tile_pool(name="pso", bufs=3, space="PSUM"))

        pp_sb = const.tile([P, DT, 8], PPDT, tag="pp")
        nc.sync.dma_start(pp_sb[:], pp_d[:])
        xt0 = xpool.tile([P, DT, TC + 1], F16, tag="xt", name="xt0")
        for dt_i in range(DT):
            nc.vector.memset(xt0[:, dt_i, 0:1], 0.0)
        nc.sync.dma_start(xt0[:, :, 1:], x_d[:, :, 0:TC])
        w_sb = {}
        for nm, dram, dt_ in (("k", wk_d, F8), ("v", wv_d, F16),
                              ("r", wr_d, F8), ("o", wo_d, F16)):
            w = const.tile([P, DT, D], dt_, tag=f"w{nm}")
            nc.sync.dma_start(w[:], dram[:])
            w_sb[nm] = w

        def pc(dt_i, j):
            return pp_sb[:, dt_i, j : j + 1]

        tte = nc.gpsimd if GPS_TT else nc.vector

        def make_mixes(xt):
            """diff (Pool-able tt) + 3 token-shift mixes (DVE stt);
            fp8 out for k/r, fp16 for v."""
            xk8 = mixp.tile([P, DT, TC], F8, tag="xk8")
            xr8 = mixp.tile([P, DT, TC], F8, tag="xr8")
            xv16 = mixp.tile([P, DT, TC], F16, tag="xv16")
            for dt_i in range(DT):
                diff = diffp.tile([P, TC], F16, tag="diff")
                tte.tensor_sub(diff[:], xt[:, dt_i, 1:], xt[:, dt_i, 0:TC])
                nc.vector.scalar_tensor_tensor(
                    xk8[:, dt_i, :], diff[:], pc(dt_i, 0), xt[:, dt_i, 0:TC],
                    AL.mult, AL.add)
                nc.vector.scalar_tensor_tensor(
                    xv16[:, dt_i, :], diff[:], pc(dt_i, 1), xt[:, dt_i, 0:TC],
                    AL.mult, AL.add)
                nc.vector.scalar_tensor_tensor(
                    xr8[:, dt_i, :], diff[:], pc(dt_i, 2), xt[:, dt_i, 0:TC],
                    AL.mult, AL.add)
            return xk8, xr8, xv16

        # persistent scan-state planes (chunk ch init reads the last column
        # written by chunk ch-1; DVE is in-order so in-place is safe)
        c_pl = pl1.tile([P, DT, TC], F16, tag="c_pl")
        n_pl = pl1.tile([P, DT, TC], F16, tag="n_pl")

        dk_mat = None
        if SCAN_PROBE:
            # materialized decay tile for dt0: is the stride-0 broadcast
            # data0 what makes hw scans ~2.4x the cost model?
            dk_mat = const.tile([P, TC], F16, tag="dk_mat")
            nc.scalar.activation(dk_mat[:], pc(0, 3).to_broadcast((P, TC)),
                                 AF.Copy)

        def dma_x(ch_i):
            xt = xpool.tile([P, DT, TC + 1], F16, tag="xt")
            nc.sync.dma_start(xt[:], x_d[:, :, ch_i * TC - 1 : (ch_i + 1) * TC])
            return xt

        mixes = make_mixes(xt0)  # chunk 0, pipeline warm-up
        xt_next = dma_x(1) if NCH > 1 else None
        pend = None

        for ch in range(NCH):
            t0 = ch * TC
            xk8, xr8, xv16 = mixes

            # two-deep prefetch: DMA ch+2 now, mix ch+1 (DMA'd last iter)
            if ch + 2 < NCH:
                xt_next2 = dma_x(ch + 2)
            else:
                xt_next2 = None
            if xt_next is not None:
                mixes = make_mixes(xt_next)
            xt_next = xt_next2

            imp = pl2.tile([P, DT, TC], F16, tag="imp")
            v16 = pl2.tile([P, DT, TC], F16, tag="v16")
            er = pl2.tile([P, DT, TC], F16, tag="er")

            # ---- K/R fp8 DoubleRow GEMMs + V fp16 GEMM
            for dt_i in range(DT):
                cs = slice(dt_i * P, (dt_i + 1) * P)

                ps_k = psp.tile([P, TC], F32, tag="ps")
                for j in range(DT // 2):
                    nc.tensor.matmul(
                        ps_k[:], w_sb["k"][:, 2 * j : 2 * j + 2, cs],
                        xk8[:, 2 * j : 2 * j + 2, :],
                        start=(j == 0), stop=(j == DT // 2 - 1), perf_mode=DR)
                nc.scalar.activation(imp[:, dt_i, :], ps_k[:], AF.Exp,
                                     scale=KSCALE)

                ps_v = psp.tile([P, TC], F32, tag="ps")
                for eo in range(DT):
                    nc.tensor.matmul(
                        ps_v[:], w_sb["v"][:, eo, cs], xv16[:, eo, :],
                        start=(eo == 0), stop=(eo == DT - 1))
                nc.scalar.copy(v16[:, dt_i, :], ps_v[:])

                ps_r = psp.tile([P, TC], F32, tag="ps")
                for j in range(DT // 2):
                    nc.tensor.matmul(
                        ps_r[:], w_sb["r"][:, 2 * j : 2 * j + 2, cs],
                        xr8[:, 2 * j : 2 * j + 2, :],
                        start=(j == 0), stop=(j == DT // 2 - 1), perf_mode=DR)
                nc.scalar.activation(er[:, dt_i, :], ps_r[:], AF.Exp,
                                     scale=-KSCALE)

            # ---- u on DVE, then drain previous chunk's tail immediately
            # (its ACT ln/exp finished last iteration)
            u = pl1.tile([P, DT, TC], F16, tag="u")
            nc.vector.tensor_mul(u[:], imp[:], v16[:])
            if pend is not None:
                _flush_tail(nc, tte, pso, outp, out_d, w_sb["o"], pend)
                pend = None

            # ACT writes the gamma-scaled terms straight into the tail
            # planes (overlapping the DVE scans): num <- gain*u,
            # den2 <- gain*imp (per-partition Copy scale)
            num = pl1.tile([P, DT, TC], F16, tag="num")
            den2 = pl1.tile([P, DT, TC], F16, tag="den2")
            for dt_i in range(DT):
                nc.scalar.mul(num[:, dt_i, :], u[:, dt_i, :], pc(dt_i, 4))
                nc.scalar.mul(den2[:, dt_i, :], imp[:, dt_i, :], pc(dt_i, 4))

            for dt_i in range(DT):
                decay_b = pc(dt_i, 3).to_broadcast((P, TC))
                init_c = 0.0 if ch == 0 else c_pl[:, dt_i, TC - 1 : TC]
                init_n = 0.0 if ch == 0 else n_pl[:, dt_i, TC - 1 : TC]
                nc.vector.tensor_tensor_scan(
                    c_pl[:, dt_i, :], decay_b, u[:, dt_i, :], init_c,
                    AL.mult, AL.add)
                nc.vector.tensor_tensor_scan(
                    n_pl[:, dt_i, :], decay_b, imp[:, dt_i, :], init_n,
                    AL.mult, AL.add)

            # big-AP tt tail (2x rate): num += c ; den2 += n (= den);
            # u := den2*er ; den2 += u  (= den*(1+er))
            nc.vector.tensor_add(num[:], num[:], c_pl[:])
            nc.vector.tensor_add(den2[:], den2[:], n_pl[:])
            nc.vector.tensor_mul(u[:], den2[:], er[:])
            nc.vector.tensor_add(den2[:], den2[:], u[:])

            # ---- batched reciprocal on ACT: den2 -> 1/den2 (in place)
            nc.scalar.activation(den2[:, :, :], den2[:, :, :], AF.Ln)
            nc.scalar.activation(den2[:, :, :], den2[:, :, :], AF.Exp,
                                 scale=-1.0)

            pend = (num, den2, ch)

        _flush_tail(nc, tte, pso, outp, out_d, w_sb["o"], pend)

    nc.compile()
    return nc


def _flush_tail(nc, tte, pso, outp, out_d, wo_sb, pend):
    num, recip, ch = pend
    TC = num.shape[2]
    t0 = ch * TC
    rwkv = outp.tile([P, DT, TC], F16, tag="rwkv")
    tte.tensor_mul(rwkv[:], num[:], recip[:])
    out16 = outp.tile([P, DT, TC], F16, tag="out16")
    for co in range(DT):
        cs = slice(co * P, (co + 1) * P)
        ps_o = pso.tile([P, TC], F32, tag="pso")
        for eo in range(DT):
            nc.tensor.matmul(
                ps_o[:], wo_sb[:, eo, cs], rwkv[:, eo, :],
                start=(eo == 0), stop=(eo == DT - 1))
        nc.scalar.copy(out16[:, co, :], ps_o[:])
    nc.sync.dma_start(out_d[:, :, t0 : t0 + TC], out16[:])


def _pack_vec(v):
    # [D] -> [P, DT]
    return np.ascontiguousarray(v.reshape(DT, P).T)


def _packw_T(W):
    # W [c, e] -> W.T [e, c] -> [P, DT, D]
    return np.ascontiguousarray(W.T.reshape(DT, P, D).transpose(1, 0, 2))


def pack_inputs(x, Wk, Wv, Wr, Wo, mix_k, mix_v, mix_r, log_gain, log_decay):
    T = x.shape[1]
    decay = np.exp(-np.exp(log_decay.astype(np.float64))).astype(np.float32)
    gain = (np.exp(log_gain.astype(np.float64)) - 1.0).astype(np.float32)
    # mix scalars are applied to fp16 x, producing fp8 (k/r) pre-scaled by
    # SX: fold SX into the mix coefficients? No -- SX folds into x itself
    # for the k/r paths via the stt scalar... the stt computes
    # (diff*mix + x_prev); scaling by SX must hit both terms, so scale x.
    # Instead: x stays unscaled fp16; the fp8 quantization step of the stt
    # output loses range for small values. Scale mix path by folding SX
    # into the *weights* being fp8 is not possible (they're also fp8).
    # Resolution: ship x pre-scaled by SX (fp16 holds 5.5*32=176 fine) and
    # de-scale in the V GEMM via the fp16 weights (Wv/SX) and in KSCALE.
    pp = np.zeros((P, DT, 8), np.float32)
    for j, v in enumerate((mix_k, mix_v, mix_r, decay, gain)):
        pp[:, :, j] = _pack_vec(v.astype(np.float32))

    wk8 = _packw_T((Wk.astype(np.float64) * SW)).astype(E4NP)
    wr8 = _packw_T((Wr.astype(np.float64) * SW)).astype(E4NP)
    # x is shipped *SX; compensate inside the fp16 V and... V reads x*SX,
    # so Wv scales by 1/SX. O reads rwkv (unscaled).
    wv16 = _packw_T((Wv.astype(np.float64) / SX)).astype(np.float16)
    wo16 = _packw_T(Wo).astype(np.float16)

    xs = (x.astype(np.float64) * SX).astype(np.float16)

    in_maps = []
    for b in range(x.shape[0]):
        xb = np.ascontiguousarray(
            xs[b].T.reshape(DT, P, T).transpose(1, 0, 2))
        in_maps.append({"x": xb, "wk": wk8, "wv": wv16, "wr": wr8,
                        "wo": wo16, "pp": pp})
    return in_maps


def unpack_output(arrs, T):
    out = np.empty((len(arrs), T, D), np.float32)
    for b, a in enumerate(arrs):
        out[b] = a.astype(np.float32).transpose(2, 1, 0).reshape(T, D)
    return out


_NC_CACHE = {}


def run(inputs, trace=False, **kw):
    x = np.asarray(inputs["x"])
    Bx, T, Dx = x.shape
    assert Dx == D and Bx == B
    key = (T, TC_DEFAULT, GPS_TT, SCAN_PROBE)
    if key not in _NC_CACHE:
        _NC_CACHE[key] = build(T=T)
    nc = _NC_CACHE[key]
    in_maps = pack_inputs(
        x,
        np.asarray(inputs["Wk"]), np.asarray(inputs["Wv"]),
        np.asarray(inputs["Wr"]), np.asarray(inputs["Wo"]),
        np.asarray(inputs["mix_k"]), np.asarray(inputs["mix_v"]),
        np.asarray(inputs["mix_r"]),
        np.asarray(inputs["log_gain"]), np.asarray(inputs["log_decay"]),
    )
    res = run_bass_kernel_spmd(nc, in_maps, core_ids=list(range(B)), trace=trace, **kw)
    out = unpack_output([res.results[i]["out"] for i in range(B)], T)
    return out, res


def kernel(**inputs):
    return run(inputs)[0]


if __name__ == "__main__":
    nc = build(T=512)
    print("built ok")


# revision 28
# speedup vs baseline: 1.3536x; 1.1702x over previous
"""RWKV-style AttentionBlock kernel for 8 Trainium2 NeuronCores (v3).

Problem: B=8, T=4096, D=1024, f32 in/out.
  per sequence: k/v/r = token-shift-mixed x @ W{k,v,r}.T ; imp = exp(k)
  WKV linear recurrence over time (per-channel decay), bonus-gain readout,
  rwkv = sigmoid(r) * wkv ; out = rwkv @ Wo.T

Sharding: pure data-parallel, one batch element per core (no collectives).

Measured engine economics (HW traces):
  - DVE: tt 413ns, stt 732ns (no fast mode), scan 1272ns per [128,512]
  - ACT: ~693ns per [128,512] op
  - PE fp16 matmul [128ctr,512]: 213ns; fp8e4 DoubleRow [256ctr,512]: ~250ns
    (i.e. DR is ~2x flops/s of fp16 -- NOT the 4x the cost model claims)
  - TRN fp8e4 is IEEE float8_e4m3: max normal 240 (not 448!)

Design:
  - K and R projections run as fp8 DoubleRow GEMMs (half PE cost); their
    quantization error survives the WKV ratio / sigmoid (~1.1e-2 total,
    gate 2e-2). V and O stay fp16 (value path is 3.7e-2 if fp8).
  - token-shift mixes produce the GEMM inputs: xk8/xr8 fp8 straight out
    of the stt (stt has no fast mode, so fp8 output is free), xv16 fp16.
  - mixes + diff optionally run on the idle GpSimd (Pool) engine
    (GPS_MIX=1) to unload the bottleneck DVE.
  - u = imp*v and rwkv = num*recip as single [128, 8*TC] fp16 tt ops;
    ln/exp reciprocal batched on ACT; scan state planes updated in place
    (DVE is in-order).
  - one-chunk software pipelining: mixes run a chunk ahead of the GEMMs;
    the rwkv mul + O GEMM of chunk ch runs during chunk ch+1.
"""

import os
import numpy as np
from contextlib import ExitStack

import ml_dtypes

import concourse.mybir as mybir
import concourse.tile as tile
from concourse import bacc
from concourse.bass_utils import run_bass_kernel_spmd

# ---------------------------------------------------------------------------
# Pin Exp/Ln to the one ACT table set holding both (avoids ~1.3us table
# reloads between exp and ln on the scalar engine).
import concourse.hw_specs as _hw_specs

_orig_get_activation_tables = _hw_specs.get_activation_tables


def _pinned_activation_tables(arch):
    tabs = _orig_get_activation_tables(arch)
    AF_ = mybir.ActivationFunctionType
    both = [n for n, fs in tabs.items() if AF_.Exp in fs and AF_.Ln in fs]
    if both:
        keep = both[0]
        for n, fs in tabs.items():
            if n != keep:
                fs.discard(AF_.Exp)
                fs.discard(AF_.Ln)
    return tabs


if os.environ.get("PIN_ACT_TABLES", "1") == "1":
    _hw_specs.get_activation_tables = _pinned_activation_tables
    bacc.get_activation_tables = _pinned_activation_tables

P = 128
D = 1024
DT = D // P          # 8 channel tiles
B = 8
T_FULL = 4096
TC_DEFAULT = 512

F16 = mybir.dt.float16
F32 = mybir.dt.float32
F8 = mybir.dt.float8e4
E4NP = ml_dtypes.float8_e4m3  # IEEE e4m3: max normal 240
PPDT = F32  # fp16 per-partition scalars deadlock the DVE on hw; keep f32
AL = mybir.AluOpType
AF = mybir.ActivationFunctionType
DR = mybir.MatmulPerfMode.DoubleRow

SX = 32.0     # x (and mixed x) scale into fp8: |x|max ~5.5 -> 176 < 240
SW = 1024.0   # weight scale into fp8: |W|max ~0.11 -> ~115 < 240
KSCALE = 1.0 / (SX * SW)

# run the pure tensor_tensor planes (diff, u, rwkv-mul) on GpSimd (Pool);
# stt (mixes) is not a valid Pool opcode on corev3, so those stay on DVE
GPS_TT = os.environ.get("GPS_TT", "0") == "1"
# probe: materialized decay tile for the dt0 scans (vs broadcast stride-0)
SCAN_PROBE = os.environ.get("SCAN_PROBE", "0") == "1"


def build(T=T_FULL, TC=TC_DEFAULT):
    assert T % TC == 0
    NCH = T // TC
    nc = bacc.Bacc("TRN2", target_bir_lowering=False, debug=False, num_devices=B)

    x_d = nc.dram_tensor("x", [P, DT, T], F16, kind="ExternalInput")
    wk_d = nc.dram_tensor("wk", [P, DT, D], F8, kind="ExternalInput")
    wv_d = nc.dram_tensor("wv", [P, DT, D], F16, kind="ExternalInput")
    wr_d = nc.dram_tensor("wr", [P, DT, D], F8, kind="ExternalInput")
    wo_d = nc.dram_tensor("wo", [P, DT, D], F16, kind="ExternalInput")
    # per-channel params, packed [128, DT, 8]: mix_k, mix_v, mix_r, decay, gain
    pp_d = nc.dram_tensor("pp", [P, DT, 8], PPDT, kind="ExternalInput")
    out_d = nc.dram_tensor("out", [P, DT, T], F16, kind="ExternalOutput")

    mixer = None  # set inside context

    with tile.TileContext(nc) as tc, ExitStack() as ctx:
        const = ctx.enter_context(tc.tile_pool(name="const", bufs=1))
        xpool = ctx.enter_context(tc.tile_pool(name="xpool", bufs=2))
        mixp = ctx.enter_context(tc.tile_pool(name="mixp", bufs=2))
        diffp = ctx.enter_context(tc.tile_pool(name="diffp", bufs=2))
        pl2 = ctx.enter_context(tc.tile_pool(name="pl2", bufs=2))
        pl1 = ctx.enter_context(tc.tile_pool(name="pl1", bufs=1))
        outp = ctx.enter_context(tc.tile_pool(name="outp", bufs=1))
        psp = ctx.enter_context(tc.tile_pool(name="psp", bufs=5, space="PSUM"))
        pso = ctx.enter_context(tc.tile_pool(name="pso", bufs=3, space="PSUM"))

        pp_sb = const.tile([P, DT, 8], PPDT, tag="pp")
        nc.sync.dma_start(pp_sb[:], pp_d[:])
        xt0 = xpool.tile([P, DT, TC + 1], F16, tag="xt", name="xt0")
        for dt_i in range(DT):
            nc.vector.memset(xt0[:, dt_i, 0:1], 0.0)
        nc.sync.dma_start(xt0[:, :, 1:], x_d[:, :, 0:TC])
        w_sb = {}
        for nm, dram, dt_ in (("k", wk_d, F8), ("v", wv_d, F16),
                              ("r", wr_d, F8), ("o", wo_d, F16)):
            w = const.tile([P, DT, D], dt_, tag=f"w{nm}")
            nc.sync.dma_start(w[:], dram[:])
            w_sb[nm] = w

        def pc(dt_i, j):
            return pp_sb[:, dt_i, j : j + 1]

        tte = nc.gpsimd if GPS_TT else nc.vector

        def make_mixes(xt):
            """diff (Pool-able tt) + 3 token-shift mixes (DVE stt);
            fp8 out for k/r, fp16 for v."""
            xk8 = mixp.tile([P, DT, TC], F8, tag="xk8")
            xr8 = mixp.tile([P, DT, TC], F8, tag="xr8")
            xv16 = mixp.tile([P, DT, TC], F16, tag="xv16")
            for dt_i in range(DT):
                diff = diffp.tile([P, TC], F16, tag="diff")
                tte.tensor_sub(diff[:], xt[:, dt_i, 1:], xt[:, dt_i, 0:TC])
                nc.vector.scalar_tensor_tensor(
                    xk8[:, dt_i, :], diff[:], pc(dt_i, 0), xt[:, dt_i, 0:TC],
                    AL.mult, AL.add)
                nc.vector.scalar_tensor_tensor(
                    xv16[:, dt_i, :], diff[:], pc(dt_i, 1), xt[:, dt_i, 0:TC],
                    AL.mult, AL.add)
                nc.vector.scalar_tensor_tensor(
                    xr8[:, dt_i, :], diff[:], pc(dt_i, 2), xt[:, dt_i, 0:TC],
                    AL.mult, AL.add)
            return xk8, xr8, xv16

        # persistent scan-state planes (chunk ch init reads the last column
        # written by chunk ch-1; DVE is in-order so in-place is safe)
        c_pl = pl1.tile([P, DT, TC], F16, tag="c_pl")
        n_pl = pl1.tile([P, DT, TC], F16, tag="n_pl")

        dk_mat = None
        if SCAN_PROBE:
            # materialized decay tile for dt0: is the stride-0 broadcast
            # data0 what makes hw scans ~2.4x the cost model?
            dk_mat = const.tile([P, TC], F16, tag="dk_mat")
            nc.scalar.activation(dk_mat[:], pc(0, 3).to_broadcast((P, TC)),
                                 AF.Copy)

        def dma_x(ch_i):
            xt = xpool.tile([P, DT, TC + 1], F16, tag="xt")
            nc.sync.dma_start(xt[:], x_d[:, :, ch_i * TC - 1 : (ch_i + 1) * TC])
            return xt

        mixes = make_mixes(xt0)  # chunk 0, pipeline warm-up
        xt_next = dma_x(1) if NCH > 1 else None
        pend = None

        for ch in range(NCH):
            t0 = ch * TC
            xk8, xr8, xv16 = mixes

            # two-deep prefetch: DMA ch+2 now, mix ch+1 (DMA'd last iter)
            if ch + 2 < NCH:
                xt_next2 = dma_x(ch + 2)
            else:
                xt_next2 = None
            if xt_next is not None:
                mixes = make_mixes(xt_next)
            xt_next = xt_next2

            imp = pl2.tile([P, DT, TC], F16, tag="imp")
            v16 = pl2.tile([P, DT, TC], F16, tag="v16")
            er = pl2.tile([P, DT, TC], F16, tag="er")

            # ---- K/R fp8 DoubleRow GEMMs + V fp16 GEMM
            for dt_i in range(DT):
                cs = slice(dt_i * P, (dt_i + 1) * P)

                ps_k = psp.tile([P, TC], F32, tag="ps")
                for j in range(DT // 2):
                    nc.tensor.matmul(
                        ps_k[:], w_sb["k"][:, 2 * j : 2 * j + 2, cs],
                        xk8[:, 2 * j : 2 * j + 2, :],
                        start=(j == 0), stop=(j == DT // 2 - 1), perf_mode=DR)
                nc.scalar.activation(imp[:, dt_i, :], ps_k[:], AF.Exp,
                                     scale=KSCALE)

                ps_v = psp.tile([P, TC], F32, tag="ps")
                for eo in range(DT):
                    nc.tensor.matmul(
                        ps_v[:], w_sb["v"][:, eo, cs], xv16[:, eo, :],
                        start=(eo == 0), stop=(eo == DT - 1))
                nc.scalar.copy(v16[:, dt_i, :], ps_v[:])

                ps_r = psp.tile([P, TC], F32, tag="ps")
                for j in range(DT // 2):
                    nc.tensor.matmul(
                        ps_r[:], w_sb["r"][:, 2 * j : 2 * j + 2, cs],
                        xr8[:, 2 * j : 2 * j + 2, :],
                        start=(j == 0), stop=(j == DT // 2 - 1), perf_mode=DR)
                nc.scalar.activation(er[:, dt_i, :], ps_r[:], AF.Exp,
                                     scale=-KSCALE)

            # ---- drain previous chunk's tail first (its ACT ln/exp is done)
            # so the PE can start the O GEMM early
            u = pl1.tile([P, DT, TC], F16, tag="u")
            num = pl1.tile([P, DT, TC], F16, tag="num")
            den2 = pl1.tile([P, DT, TC], F16, tag="den2")
            if pend is not None:
                _flush_tail(nc, tte, pso, outp, out_d, w_sb["o"], pend)
                pend = None

            # per-dt u + scans: the DVE starts scanning dt0 as soon as its
            # drains land instead of waiting for the whole chunk; ACT
            # gamma-scales (num <- gain*u, den2 <- gain*imp) follow each dt
            for dt_i in range(DT):
                nc.vector.tensor_mul(u[:, dt_i, :], imp[:, dt_i, :],
                                     v16[:, dt_i, :])
                decay_b = pc(dt_i, 3).to_broadcast((P, TC))
                init_c = 0.0 if ch == 0 else c_pl[:, dt_i, TC - 1 : TC]
                init_n = 0.0 if ch == 0 else n_pl[:, dt_i, TC - 1 : TC]
                nc.vector.tensor_tensor_scan(
                    c_pl[:, dt_i, :], decay_b, u[:, dt_i, :], init_c,
                    AL.mult, AL.add)
                nc.vector.tensor_tensor_scan(
                    n_pl[:, dt_i, :], decay_b, imp[:, dt_i, :], init_n,
                    AL.mult, AL.add)
                nc.scalar.mul(num[:, dt_i, :], u[:, dt_i, :], pc(dt_i, 4))
                nc.scalar.mul(den2[:, dt_i, :], imp[:, dt_i, :], pc(dt_i, 4))

            # big-AP tt tail (2x rate): num += c ; den2 += n (= den);
            # u := den2*er ; den2 += u  (= den*(1+er))
            nc.vector.tensor_add(num[:], num[:], c_pl[:])
            nc.vector.tensor_add(den2[:], den2[:], n_pl[:])
            nc.vector.tensor_mul(u[:], den2[:], er[:])
            nc.vector.tensor_add(den2[:], den2[:], u[:])

            # ---- batched reciprocal on ACT: den2 -> 1/den2 (in place)
            nc.scalar.activation(den2[:, :, :], den2[:, :, :], AF.Ln)
            nc.scalar.activation(den2[:, :, :], den2[:, :, :], AF.Exp,
                                 scale=-1.0)

            pend = (num, den2, ch)

        _flush_tail(nc, tte, pso, outp, out_d, w_sb["o"], pend)

    nc.compile()
    return nc


def _flush_tail(nc, tte, pso, outp, out_d, wo_sb, pend):
    num, recip, ch = pend
    TC = num.shape[2]
    t0 = ch * TC
    rwkv = outp.tile([P, DT, TC], F16, tag="rwkv")
    tte.tensor_mul(rwkv[:], num[:], recip[:])
    out16 = outp.tile([P, DT, TC], F16, tag="out16")
    for co in range(DT):
        cs = slice(co * P, (co + 1) * P)
        ps_o = pso.tile([P, TC], F32, tag="pso")
        for eo in range(DT):
            nc.tensor.matmul(
                ps_o[:], wo_sb[:, eo, cs], rwkv[:, eo, :],
                start=(eo == 0), stop=(eo == DT - 1))
        nc.scalar.copy(out16[:, co, :], ps_o[:])
    nc.sync.dma_start(out_d[:, :, t0 : t0 + TC], out16[:])


def _pack_vec(v):
    # [D] -> [P, DT]
    return np.ascontiguousarray(v.reshape(DT, P).T)


def _packw_T(W):
    # W [c, e] -> W.T [e, c] -> [P, DT, D]
    return np.ascontiguousarray(W.T.reshape(DT, P, D).transpose(1, 0, 2))


def pack_inputs(x, Wk, Wv, Wr, Wo, mix_k, mix_v, mix_r, log_gain, log_decay):
    T = x.shape[1]
    decay = np.exp(-np.exp(log_decay.astype(np.float64))).astype(np.float32)
    gain = (np.exp(log_gain.astype(np.float64)) - 1.0).astype(np.float32)
    # mix scalars are applied to fp16 x, producing fp8 (k/r) pre-scaled by
    # SX: fold SX into the mix coefficients? No -- SX folds into x itself
    # for the k/r paths via the stt scalar... the stt computes
    # (diff*mix + x_prev); scaling by SX must hit both terms, so scale x.
    # Instead: x stays unscaled fp16; the fp8 quantization step of the stt
    # output loses range for small values. Scale mix path by folding SX
    # into the *weights* being fp8 is not possible (they're also fp8).
    # Resolution: ship x pre-scaled by SX (fp16 holds 5.5*32=176 fine) and
    # de-scale in the V GEMM via the fp16 weights (Wv/SX) and in KSCALE.
    pp = np.zeros((P, DT, 8), np.float32)
    for j, v in enumerate((mix_k, mix_v, mix_r, decay, gain)):
        pp[:, :, j] = _pack_vec(v.astype(np.float32))

    wk8 = _packw_T((Wk.astype(np.float64) * SW)).astype(E4NP)
    wr8 = _packw_T((Wr.astype(np.float64) * SW)).astype(E4NP)
    # x is shipped *SX; compensate inside the fp16 V and... V reads x*SX,
    # so Wv scales by 1/SX. O reads rwkv (unscaled).
    wv16 = _packw_T((Wv.astype(np.float64) / SX)).astype(np.float16)
    wo16 = _packw_T(Wo).astype(np.float16)

    xs = (x.astype(np.float64) * SX).astype(np.float16)

    in_maps = []
    for b in range(x.shape[0]):
        xb = np.ascontiguousarray(
            xs[b].T.reshape(DT, P, T).transpose(1, 0, 2))
        in_maps.append({"x": xb, "wk": wk8, "wv": wv16, "wr": wr8,
                        "wo": wo16, "pp": pp})
    return in_maps


def unpack_output(arrs, T):
    out = np.empty((len(arrs), T, D), np.float32)
    for b, a in enumerate(arrs):
        out[b] = a.astype(np.float32).transpose(2, 1, 0).reshape(T, D)
    return out


_NC_CACHE = {}


def run(inputs, trace=False, **kw):
    x = np.asarray(inputs["x"])
    Bx, T, Dx = x.shape
    assert Dx == D and Bx == B
    key = (T, TC_DEFAULT, GPS_TT, SCAN_PROBE)
    if key not in _NC_CACHE:
        _NC_CACHE[key] = build(T=T)
    nc = _NC_CACHE[key]
    in_maps = pack_inputs(
        x,
        np.asarray(inputs["Wk"]), np.asarray(inputs["Wv"]),
        np.asarray(inputs["Wr"]), np.asarray(inputs["Wo"]),
        np.asarray(inputs["mix_k"]), np.asarray(inputs["mix_v"]),
        np.asarray(inputs["mix_r"]),
        np.asarray(inputs["log_gain"]), np.asarray(inputs["log_decay"]),
    )
    res = run_bass_kernel_spmd(nc, in_maps, core_ids=list(range(B)), trace=trace, **kw)
    out = unpack_output([res.results[i]["out"] for i in range(B)], T)
    return out, res


def kernel(**inputs):
    return run(inputs)[0]


if __name__ == "__main__":
    nc = build(T=512)
    print("built ok")
